# revision 68
# baseline (speedup 1.0000x reference)
"""Trainium2 Bass kernel for nn_AttentionCell (sliding-window attention).

Reference computation (per batch b):
    contexts[t, l] = symbols[clip(t - 15 + l, 0)]      l in [0, 16)
    scores[t, l]   = M[contexts[t, l]] . enc[t]
    p              = softmax_l(scores)
    compressed[t]  = sum_l p[t, l] * C[contexts[t, l]]
    out            = concat([compressed, enc], -1)
    returns (out, p)

Key identities used:
  * With sym_pad[j] = symbols[clip(j - 15, 0, T-1)] (length T+15), the
    context symbol for (t, l) is sym_pad[t + l]; so this is banded
    attention with keys K[j] = M[sym_pad[j]], values Vv[j] = C[sym_pad[j]].
  * K^T is gathered directly in the transposed layout the scores matmul
    needs via a one-hot matmul: K^T = matmul(lhsT=M, rhs=OH).
  * Vv is gathered in natural layout via matmul(lhsT=OH_slice, rhs=C).
  * The softmax band (the 16 diagonals of each 128x143 score tile) is
    extracted with a batched DRAM round trip per 4-chunk group: exp rows
    written contiguously make each chunk's diagonal a regular
    [[4*143+1, 128], [143, 4], [1, 16]] strided read.

Sharding: data-parallel over batch, one batch element per NeuronCore (8).
"""

import numpy as np
import ml_dtypes

BF16 = ml_dtypes.bfloat16

B, T, V, D, L = 8, 2048, 256, 512, 16
TP = T + L - 1            # 2063: padded symbol/key length
NCH = T // 128            # 16 query chunks of 128
SPAN = 128 + L - 1        # 143: key span per chunk
N_CORES = 8

_NC_CACHE = {}


def _patch_tile_drain():
    """The stock walrus in this toolchain rejects instructions carrying more
    than one semaphore wait ("Too many sync wait commands"), but Tile's final
    drain aggregates one wait per active logical processor. Split that drain
    into one drain per processor (one wait each)."""
    import concourse.tile as tile
    from concourse.vector_clock import ScopedClock, VectorClock

    if getattr(tile.TileContext, "_ant_drain_patched", False):
        return

    def _drain_and_barrier(self, tick_clock, wait_clock):
        gc = tick_clock.global_clock
        n = len(gc)
        for proc in range(n):
            tick = gc[proc]
            if tick <= 0:
                continue
            vc = VectorClock([0] * n)
            vc.require_at_least(proc, tick)
            d = self.nc.sync.drain()
            wait_clock.add_sem_waits(d.ins, ScopedClock({None: vc}))
        self.nc.all_engine_barrier()
        assert self.sems is not None
        popped = self.nc._tile_sem_poison_stack.pop()
        assert popped is self._sem_poison
        self.nc.clear_and_free_semaphores(list(self.sems.allocated().values()))
        self.nc.all_engine_barrier()

    tile.TileContext._drain_and_barrier = _drain_and_barrier
    tile.TileContext._ant_drain_patched = True


_MAX_WAITS = 1  # stock walrus rejects instructions with more sem waits
_MAX_WAITS_BY_OP = {}


def _fix_excess_waits(nc):
    """Hoist semaphore waits beyond the per-instruction limit onto injected
    EventSemaphore instructions placed immediately before, on the same engine
    queue. Returns patched BIR JSON bytes and pins them on nc.to_json_bytes."""
    import orjson

    bir = orjson.loads(nc.to_json_bytes())
    n_fix = 0
    for func in bir["functions"]:
        for blk in func["blocks"]:
            new_insts = []
            for inst in blk["instructions"]:
                si = inst.get("sync_info") or {}
                waits = si.get("on_wait") or []
                maxw = _MAX_WAITS_BY_OP.get(inst["opcode"], _MAX_WAITS)
                if len(waits) > maxw:
                    excess = waits[:-maxw]
                    si["on_wait"] = waits[-maxw:]
                    for i in range(0, len(excess), _MAX_WAITS):
                        chunk = excess[i:i + _MAX_WAITS]
                        new_insts.append(
                            {
                                "debug": inst.get("debug", 0),
                                "engine": inst["engine"],
                                "ins": [],
                                "name": f"antwaitfix_{n_fix}_{i}",
                                "opcode": "EventSemaphore",
                                "outs": [],
                                "sync_info": {"on_update": [], "on_wait": chunk},
                            }
                        )
                    n_fix += 1
                new_insts.append(inst)
            blk["instructions"] = new_insts
    patched = orjson.dumps(bir)
    nc.to_json_bytes = lambda: patched
    return patched


def _build_nc():
    from contextlib import ExitStack

    import concourse.bass as bass
    import concourse.tile as tile
    from concourse import mybir

    _patch_tile_drain()

    f32 = mybir.dt.float32
    bf16 = mybir.dt.bfloat16
    Copy = mybir.ActivationFunctionType.Copy
    Exp = mybir.ActivationFunctionType.Exp
    AX = mybir.AxisListType.X

    nc = bass.Bass()
    enc = nc.declare_dram_parameter("enc", [T, D], f32, isOutput=False)
    # packed bf16 consts: per v-chunk rows, cols = [M | C | ident_b]
    # ident_b occupies cols [2*D : 2*D+128] of v-chunk 0 only.
    BW = 2 * D + 128
    cb = nc.declare_dram_parameter("cb", [2, 128, BW], bf16, isOutput=False)
    # packed f32 consts: cols = [ident_f | maskneg]
    FW = 128 + SPAN
    cf = nc.declare_dram_parameter("cf", [128, FW], f32, isOutput=False)
    oh = nc.declare_dram_parameter("oh", [2, 128, TP], bf16, isOutput=False)
    out = nc.declare_dram_parameter("out", [T, 2 * D], f32, isOutput=True)
    p_out = nc.declare_dram_parameter("p", [T, L], f32, isOutput=True)

    GSZ = 4  # chunks per pipeline group

    # DRAM views with the 128-query chunk dim explicit
    enc_v = enc[:, :].rearrange("(n p) d -> p n d", p=128)       # [128,16,512]
    out_v = out[:, :].rearrange("(n p) c -> p n c", p=128)       # [128,16,1024]
    p_v = p_out[:, :].rearrange("(n p) l -> p n l", p=128)       # [128,16,16]

    with tile.TileContext(nc) as tc, ExitStack() as ctx:
        consts = ctx.enter_context(tc.tile_pool(name="consts", bufs=1))
        big = ctx.enter_context(tc.tile_pool(name="big", bufs=1))
        kt_pool = ctx.enter_context(tc.tile_pool(name="ktp", bufs=1))
        vv_pool = ctx.enter_context(tc.tile_pool(name="vvp", bufs=1))
        work = ctx.enter_context(tc.tile_pool(name="work", bufs=2))
        pbig = ctx.enter_context(tc.tile_pool(name="pbig", bufs=2, space="PSUM"))
        pqt = ctx.enter_context(tc.tile_pool(name="pqt", bufs=2, space="PSUM"))
        ps_pool = ctx.enter_context(tc.tile_pool(name="psc", bufs=2, space="PSUM"))
        pet = ctx.enter_context(tc.tile_pool(name="pet", bufs=2, space="PSUM"))
        scr_pool = ctx.enter_context(tc.tile_pool(name="scr", bufs=1, space="DRAM"))

        # ---- constants into SBUF (small DMAs first to unblock compute) ----
        cf_sb = consts.tile([128, FW], f32, tag="cf", name="cf_sb")
        nc.sync.dma_start(out=cf_sb, in_=cf[:, :])
        cb_sb = consts.tile([128, 2, BW], bf16, tag="cb", name="cb_sb")
        nc.sync.dma_start(out=cb_sb, in_=cb[:, :, :].rearrange("v p c -> p v c"))
        # one-hot, split by t-block so the first gathers start early
        oh_t = consts.tile([128, 2, TP], bf16, tag="oh", name="oh_t")
        oh_v = oh[:, :, :].rearrange("v p c -> p v c")
        _oh_parts = []
        for tb in range(5):
            c0 = 512 * tb
            cw = min(512, TP - c0)
            _oh_parts.append((c0, cw))
        nc.sync.dma_start(
            out=oh_t[:, :, 0:512], in_=oh_v[:, :, 0:512]
        )

        m_sb = [cb_sb[:, v, 0:D] for v in range(2)]
        c_sb = [cb_sb[:, v, D:2 * D] for v in range(2)]
        idb_sb = cb_sb[:, 0, 2 * D:2 * D + 128]
        idf_sb = cf_sb[:, 0:128]
        msk_sb = cf_sb[:, 128:128 + SPAN]
        oh_sb = [oh_t[:, v, :] for v in range(2)]

        # ---- persistent big buffers ----
        enc_all = big.tile([128, NCH, D], f32, tag="enc_all", name="enc_all")
        comp_all = big.tile([128, NCH, D], f32, tag="comp_all", name="comp_all")
        e_all = big.tile([128, NCH, SPAN], bf16, tag="e_all", name="e_all")
        band_all = big.tile([128, NCH, L], bf16, tag="band_all", name="band_all")
        p_all = big.tile([128, NCH, L], f32, tag="p_all", name="p_all")
        den_all = big.tile([128, NCH], f32, tag="den_all", name="den_all")
        inv_all = big.tile([128, NCH], f32, tag="inv_all", name="inv_all")

        # load encodings in 4 chunked DMAs, interleaved with the remaining
        # one-hot blocks (FIFO order tuned so chunk-0 work starts earliest)
        for h in range(2):
            nc.sync.dma_start(
                out=enc_all[:, 2 * h:2 * (h + 1), :],
                in_=enc_v[:, 2 * h:2 * (h + 1), :],
            )
        for c0, cw in _oh_parts[1:]:
            nc.sync.dma_start(
                out=oh_t[:, :, c0:c0 + cw], in_=oh_v[:, :, c0:c0 + cw]
            )
        for h in range(2, 8):
            nc.sync.dma_start(
                out=enc_all[:, 2 * h:2 * (h + 1), :],
                in_=enc_v[:, 2 * h:2 * (h + 1), :],
            )

        # ---- gathers via one-hot matmul, emitted interleaved with the
        # group pipeline so each group's inputs come first in queue order ----
        kt_sb = [
            kt_pool.tile([128, TP], bf16, tag=f"kt{d}", name=f"kt_sb{d}")
            for d in range(4)
        ]
        vv_sb = [
            vv_pool.tile([128, D], bf16, tag=f"vv{j}", name=f"vv_sb{j}")
            for j in range(17)
        ]

        def emit_kt_block(tb):
            c0 = 512 * tb
            cw = min(512, TP - c0)
            for d in range(4):
                kps = pbig.tile([128, 512], f32, tag="pbig", name="kps")
                for v in range(2):
                    nc.tensor.matmul(
                        kps[:, :cw],
                        lhsT=m_sb[v][:, 128 * d:128 * (d + 1)],
                        rhs=oh_sb[v][:, c0:c0 + cw],
                        start=(v == 0),
                        stop=(v == 1),
                    )
                nc.scalar.activation(
                    out=kt_sb[d][:, c0:c0 + cw], in_=kps[:, :cw], func=Copy
                )

        def emit_vv_tile(j):
            r0 = 128 * j
            rw = min(128, TP - r0)  # 128, last tile 15
            vps = pbig.tile([128, 512], f32, tag="pbig", name="vps")
            for v in range(2):
                nc.tensor.matmul(
                    vps[:rw, :],
                    lhsT=oh_sb[v][:, r0:r0 + rw],
                    rhs=c_sb[v],
                    start=(v == 0),
                    stop=(v == 1),
                )
            nc.vector.tensor_copy(out=vv_sb[j][:rw, :], in_=vps[:rw, :])

        # a group covering chunks [g0, g0+gs) needs KT t-blocks up to
        # (128*(g0+gs)+142)//512 and Vv tiles up to g0+gs. Pre-emit group
        # 0's needs; trickle the rest a group ahead.
        _kt_done = [0]
        _vv_done = [0]

        def need_for_chunks(hi):
            kt_hi = min((128 * hi + SPAN - 1) // 512 + 1, 5)
            vv_hi = min(hi + 1, 17)
            while _kt_done[0] < kt_hi:
                emit_kt_block(_kt_done[0])
                _kt_done[0] += 1
            while _vv_done[0] < vv_hi:
                emit_vv_tile(_vv_done[0])
                _vv_done[0] += 1

        # ---- main pipeline: variable-size groups (finer at the tail) ----
        GROUPS = [(0, 4), (4, 4), (8, 4), (12, 4)]
        NG = len(GROUPS)
        need_for_chunks(GROUPS[0][0] + GROUPS[0][1])

        def emit_phase_a(g0_, gs_):
            for k in range(g0_, g0_ + gs_):
                t0 = 128 * k
                # enc -> bf16 on the otherwise-idle GPSIMD engine, then
                # PE-transpose at the bf16 rate (half the fp32 cost)
                qb_t = work.tile([128, 512], bf16, tag="qb", name="qb_t")
                nc.gpsimd.tensor_copy(out=qb_t, in_=enc_all[:, k, :])
                qps = pqt.tile([128, 512], bf16, tag="pqt", name="qps")
                for d in range(4):
                    nc.tensor.transpose(
                        out=qps[:, 128 * d:128 * (d + 1)],
                        in_=qb_t[:, 128 * d:128 * (d + 1)],
                        identity=idb_sb,
                    )
                qt_t = work.tile([128, 512], bf16, tag="qt", name="qt_t")
                nc.vector.tensor_copy(out=qt_t, in_=qps)

                # scores S[t, c] = sum_d QT[d, t] * KT[d, t0 + c]
                sps = ps_pool.tile([128, SPAN], f32, tag="ps", name="sps")
                for d in range(4):
                    nc.tensor.matmul(
                        sps,
                        lhsT=qt_t[:, 128 * d:128 * (d + 1)],
                        rhs=kt_sb[d][:, t0:t0 + SPAN],
                        start=(d == 0),
                        stop=(d == 3),
                    )
                # band mask (0 in band, -30000 outside), then exp -> bf16
                sm_t = work.tile([128, SPAN], f32, tag="sm", name="sm_t")
                nc.vector.tensor_add(out=sm_t, in0=sps, in1=msk_sb)
                nc.scalar.activation(out=e_all[:, k, :], in_=sm_t, func=Exp)

        # software-pipelined emission: phase A runs two groups ahead of B/C
        need_for_chunks(GROUPS[1][0] + GROUPS[1][1])
        emit_phase_a(*GROUPS[0])
        emit_phase_a(*GROUPS[1])
        for g, (g0, gs) in enumerate(GROUPS):
            ks = range(g0, g0 + gs)
            if g + 2 < NG:
                need_for_chunks(GROUPS[g + 2][0] + GROUPS[g + 2][1])
                emit_phase_a(*GROUPS[g + 2])

            # phase B: batched band extraction + softmax normalizers.
            # Rows are packed contiguously (stride SPAN); the diagonal of
            # each chunk tile is then a regular [[gs*SPAN+1],[SPAN],[1]] read
            # (band[p, c, l] sits at p*(gs*SPAN) + c*SPAN + p + l).
            scr_t = scr_pool.tile(
                [128 * GSZ * SPAN], bf16, tag="scr", name="scr_t", bufs=2
            )
            scr_ap = scr_t[:]
            scr_w_ap = bass.AP(
                tensor=scr_ap.tensor,
                offset=scr_ap.offset,
                ap=[[gs * SPAN, 128], [SPAN, gs], [1, SPAN]],
            )
            nc.sync.dma_start(
                out=scr_w_ap, in_=e_all[:, g0:g0 + gs, :]
            )
            band_src = bass.AP(
                tensor=scr_ap.tensor,
                offset=scr_ap.offset,
                ap=[[gs * SPAN + 1, 128], [SPAN, gs], [1, L]],
            )
            nc.scalar.dma_start(
                out=band_all[:, g0:g0 + gs, :], in_=band_src
            )
            nc.vector.reduce_sum(
                out=den_all[:, g0:g0 + gs],
                in_=band_all[:, g0:g0 + gs, :],
                axis=AX,
            )
            nc.vector.reciprocal(
                out=inv_all[:, g0:g0 + gs],
                in_=den_all[:, g0:g0 + gs],
            )
            for k in ks:
                nc.vector.tensor_scalar_mul(
                    out=p_all[:, k, :], in0=band_all[:, k, :],
                    scalar1=inv_all[:, k:k + 1],
                )
            nc.sync.dma_start(
                out=p_v[:, g0:g0 + gs, :],
                in_=p_all[:, g0:g0 + gs, :],
            )

            # phase C: E^T transpose + value matmul per chunk
            for k in ks:
                eps_t = pet.tile([128, 256], bf16, tag="pet", name="eps_t")
                nc.tensor.transpose(
                    out=eps_t[:, 0:128], in_=e_all[:, k, 0:128], identity=idb_sb
                )
                nc.tensor.transpose(
                    out=eps_t[0:15, 128:256], in_=e_all[:, k, 128:SPAN],
                    identity=idb_sb,
                )
                eta_t = work.tile([128, 128], bf16, tag="eta", name="eta_t")
                nc.vector.tensor_copy(out=eta_t, in_=eps_t[:, 0:128])
                etb_t = work.tile([15, 128], bf16, tag="etb", name="etb_t")
                nc.scalar.activation(
                    out=etb_t, in_=eps_t[0:15, 128:256], func=Copy
                )

                # numerator[t, :] = sum_c E[t, c] * Vv[t0 + c, :]
                nps = pbig.tile([128, 512], f32, tag="pbig", name="nps")
                nc.tensor.matmul(
                    nps, lhsT=eta_t, rhs=vv_sb[k], start=True, stop=False
                )
                nc.tensor.matmul(
                    nps, lhsT=etb_t, rhs=vv_sb[k + 1][0:15, :],
                    start=False, stop=True,
                )
                nc.scalar.activation(
                    out=comp_all[:, k, :], in_=nps, func=Copy,
                    scale=inv_all[:, k:k + 1],
                )

            # compressed half of the output for this group; finer writes in
            # the last groups shrink the drain tail
            step = 1
            for lo in range(g0, g0 + gs, step):
                nc.sync.dma_start(
                    out=out_v[:, lo:lo + step, 0:D],
                    in_=comp_all[:, lo:lo + step, :],
                )
            # passthrough half for this group (no consumers; fills DMA
            # slack; split so it never blocks a latency-critical DMA long)
            for lo in range(g0, g0 + gs):
                nc.sync.dma_start(
                    out=out_v[:, lo:lo + 1, D:2 * D],
                    in_=enc_all[:, lo:lo + 1, :],
                )

    _fix_excess_waits(nc)
    return nc


def _host_inputs(symbols, encodings, M, C):
    BW = 2 * D + 128
    FW = 128 + SPAN
    Mb = M.astype(BF16)
    Cb = C.astype(BF16)
    tt = np.arange(128)[:, None]
    cc = np.arange(SPAN)[None, :]
    mask = np.where((cc - tt >= 0) & (cc - tt <= L - 1), 0.0, -30000.0).astype(
        np.float32
    )
    cf = np.zeros((128, FW), dtype=np.float32)
    cf[:, 0:128] = np.eye(128, dtype=np.float32)
    cf[:, 128:128 + SPAN] = mask

    cb = np.zeros((2, 128, BW), dtype=BF16)
    for v in range(2):
        cb[v, :, 0:D] = Mb[128 * v:128 * (v + 1)]
        cb[v, :, D:2 * D] = Cb[128 * v:128 * (v + 1)]
    cb[0, :, 2 * D:2 * D + 128] = np.eye(128, dtype=BF16)

    pad_idx = np.clip(np.arange(TP) - (L - 1), 0, T - 1)
    vids = np.arange(V)[:, None]
    in_maps = []
    for b in range(N_CORES):
        sym_pad = np.asarray(symbols[b])[pad_idx]
        ohb = (sym_pad[None, :] == vids).astype(BF16).reshape(2, 128, TP)
        in_maps.append(
            {
                "enc": np.ascontiguousarray(encodings[b], dtype=np.float32),
                "cb": cb,
                "cf": cf,
                "oh": ohb,
            }
        )
    return in_maps


def kernel(symbols, encodings, M, C, _trace=False):
    from concourse.bass_utils import run_bass_kernel_spmd

    symbols = np.asarray(symbols)
    encodings = np.asarray(encodings)
    M = np.asarray(M, dtype=np.float32)
    C = np.asarray(C, dtype=np.float32)

    if "nc" not in _NC_CACHE:
        _NC_CACHE["nc"] = _build_nc()
    nc = _NC_CACHE["nc"]

    in_maps = _host_inputs(symbols, encodings, M, C)
    res = run_bass_kernel_spmd(
        nc, in_maps, list(range(N_CORES)), trace=_trace
    )
    out = np.stack([res.results[b]["out"] for b in range(N_CORES)]).astype(
        np.float32
    )
    p = np.stack([res.results[b]["p"] for b in range(N_CORES)]).astype(np.float32)
    if _trace:
        return (out, p), res
    return out, p


# revision 69
# speedup vs baseline: 1.0141x; 1.0141x over previous
"""Trainium2 Bass kernel for nn_AttentionCell (sliding-window attention).

Reference computation (per batch b):
    contexts[t, l] = symbols[clip(t - 15 + l, 0)]      l in [0, 16)
    scores[t, l]   = M[contexts[t, l]] . enc[t]
    p              = softmax_l(scores)
    compressed[t]  = sum_l p[t, l] * C[contexts[t, l]]
    out            = concat([compressed, enc], -1)
    returns (out, p)

Key identities used:
  * With sym_pad[j] = symbols[clip(j - 15, 0, T-1)] (length T+15), the
    context symbol for (t, l) is sym_pad[t + l]; so this is banded
    attention with keys K[j] = M[sym_pad[j]], values Vv[j] = C[sym_pad[j]].
  * K^T is gathered directly in the transposed layout the scores matmul
    needs via a one-hot matmul: K^T = matmul(lhsT=M, rhs=OH).
  * Vv is gathered in natural layout via matmul(lhsT=OH_slice, rhs=C).
  * The softmax band (the 16 diagonals of each 128x143 score tile) is
    extracted with a batched DRAM round trip per 4-chunk group: exp rows
    written contiguously make each chunk's diagonal a regular
    [[4*143+1, 128], [143, 4], [1, 16]] strided read.

Sharding: data-parallel over batch, one batch element per NeuronCore (8).
"""

import numpy as np
import ml_dtypes

BF16 = ml_dtypes.bfloat16

B, T, V, D, L = 8, 2048, 256, 512, 16
TP = T + L - 1            # 2063: padded symbol/key length
NCH = T // 128            # 16 query chunks of 128
SPAN = 128 + L - 1        # 143: key span per chunk
N_CORES = 8

_NC_CACHE = {}


def _patch_tile_drain():
    """The stock walrus in this toolchain rejects instructions carrying more
    than one semaphore wait ("Too many sync wait commands"), but Tile's final
    drain aggregates one wait per active logical processor. Split that drain
    into one drain per processor (one wait each)."""
    import concourse.tile as tile
    from concourse.vector_clock import ScopedClock, VectorClock

    if getattr(tile.TileContext, "_ant_drain_patched", False):
        return

    def _drain_and_barrier(self, tick_clock, wait_clock):
        gc = tick_clock.global_clock
        n = len(gc)
        for proc in range(n):
            tick = gc[proc]
            if tick <= 0:
                continue
            vc = VectorClock([0] * n)
            vc.require_at_least(proc, tick)
            d = self.nc.sync.drain()
            wait_clock.add_sem_waits(d.ins, ScopedClock({None: vc}))
        self.nc.all_engine_barrier()
        assert self.sems is not None
        popped = self.nc._tile_sem_poison_stack.pop()
        assert popped is self._sem_poison
        self.nc.clear_and_free_semaphores(list(self.sems.allocated().values()))
        self.nc.all_engine_barrier()

    tile.TileContext._drain_and_barrier = _drain_and_barrier
    tile.TileContext._ant_drain_patched = True


_MAX_WAITS = 1  # stock walrus rejects instructions with more sem waits
_MAX_WAITS_BY_OP = {}


def _fix_excess_waits(nc):
    """Hoist semaphore waits beyond the per-instruction limit onto injected
    EventSemaphore instructions placed immediately before, on the same engine
    queue. Returns patched BIR JSON bytes and pins them on nc.to_json_bytes."""
    import orjson

    bir = orjson.loads(nc.to_json_bytes())
    n_fix = 0
    for func in bir["functions"]:
        for blk in func["blocks"]:
            new_insts = []
            for inst in blk["instructions"]:
                si = inst.get("sync_info") or {}
                waits = si.get("on_wait") or []
                maxw = _MAX_WAITS_BY_OP.get(inst["opcode"], _MAX_WAITS)
                if len(waits) > maxw:
                    excess = waits[:-maxw]
                    si["on_wait"] = waits[-maxw:]
                    for i in range(0, len(excess), _MAX_WAITS):
                        chunk = excess[i:i + _MAX_WAITS]
                        new_insts.append(
                            {
                                "debug": inst.get("debug", 0),
                                "engine": inst["engine"],
                                "ins": [],
                                "name": f"antwaitfix_{n_fix}_{i}",
                                "opcode": "EventSemaphore",
                                "outs": [],
                                "sync_info": {"on_update": [], "on_wait": chunk},
                            }
                        )
                    n_fix += 1
                new_insts.append(inst)
            blk["instructions"] = new_insts
    patched = orjson.dumps(bir)
    nc.to_json_bytes = lambda: patched
    return patched


def _build_nc():
    from contextlib import ExitStack

    import concourse.bass as bass
    import concourse.tile as tile
    from concourse import mybir

    _patch_tile_drain()

    f32 = mybir.dt.float32
    bf16 = mybir.dt.bfloat16
    Copy = mybir.ActivationFunctionType.Copy
    Exp = mybir.ActivationFunctionType.Exp
    AX = mybir.AxisListType.X

    nc = bass.Bass()
    enc = nc.declare_dram_parameter("enc", [T, D], f32, isOutput=False)
    # packed bf16 consts: per v-chunk rows, cols = [M | C | ident_b]
    # ident_b occupies cols [2*D : 2*D+128] of v-chunk 0 only.
    BW = 2 * D + 128
    cb = nc.declare_dram_parameter("cb", [2, 128, BW], bf16, isOutput=False)
    # packed f32 consts: cols = [ident_f | maskneg]
    FW = 128 + SPAN
    cf = nc.declare_dram_parameter("cf", [128, FW], f32, isOutput=False)
    oh = nc.declare_dram_parameter("oh", [2, 128, TP], bf16, isOutput=False)
    out = nc.declare_dram_parameter("out", [T, 2 * D], f32, isOutput=True)
    p_out = nc.declare_dram_parameter("p", [T, L], f32, isOutput=True)

    GSZ = 4  # chunks per pipeline group

    # DRAM views with the 128-query chunk dim explicit
    enc_v = enc[:, :].rearrange("(n p) d -> p n d", p=128)       # [128,16,512]
    out_v = out[:, :].rearrange("(n p) c -> p n c", p=128)       # [128,16,1024]
    p_v = p_out[:, :].rearrange("(n p) l -> p n l", p=128)       # [128,16,16]

    with tile.TileContext(nc) as tc, ExitStack() as ctx:
        consts = ctx.enter_context(tc.tile_pool(name="consts", bufs=1))
        big = ctx.enter_context(tc.tile_pool(name="big", bufs=1))
        kt_pool = ctx.enter_context(tc.tile_pool(name="ktp", bufs=1))
        vv_pool = ctx.enter_context(tc.tile_pool(name="vvp", bufs=1))
        work = ctx.enter_context(tc.tile_pool(name="work", bufs=2))
        pbig = ctx.enter_context(tc.tile_pool(name="pbig", bufs=2, space="PSUM"))
        pqt = ctx.enter_context(tc.tile_pool(name="pqt", bufs=2, space="PSUM"))
        ps_pool = ctx.enter_context(tc.tile_pool(name="psc", bufs=2, space="PSUM"))
        pet = ctx.enter_context(tc.tile_pool(name="pet", bufs=2, space="PSUM"))
        scr_pool = ctx.enter_context(tc.tile_pool(name="scr", bufs=1, space="DRAM"))

        # ---- constants into SBUF (small DMAs first to unblock compute) ----
        cf_sb = consts.tile([128, FW], f32, tag="cf", name="cf_sb")
        nc.sync.dma_start(out=cf_sb, in_=cf[:, :])
        cb_sb = consts.tile([128, 2, BW], bf16, tag="cb", name="cb_sb")
        nc.sync.dma_start(out=cb_sb, in_=cb[:, :, :].rearrange("v p c -> p v c"))
        # one-hot, split by t-block so the first gathers start early
        oh_t = consts.tile([128, 2, TP], bf16, tag="oh", name="oh_t")
        oh_v = oh[:, :, :].rearrange("v p c -> p v c")
        _oh_parts = []
        for tb in range(5):
            c0 = 512 * tb
            cw = min(512, TP - c0)
            _oh_parts.append((c0, cw))
        nc.sync.dma_start(
            out=oh_t[:, :, 0:512], in_=oh_v[:, :, 0:512]
        )

        m_sb = [cb_sb[:, v, 0:D] for v in range(2)]
        c_sb = [cb_sb[:, v, D:2 * D] for v in range(2)]
        idb_sb = cb_sb[:, 0, 2 * D:2 * D + 128]
        idf_sb = cf_sb[:, 0:128]
        msk_sb = cf_sb[:, 128:128 + SPAN]
        oh_sb = [oh_t[:, v, :] for v in range(2)]

        # ---- persistent big buffers ----
        enc_all = big.tile([128, NCH, D], f32, tag="enc_all", name="enc_all")
        comp_all = big.tile([128, NCH, D], f32, tag="comp_all", name="comp_all")
        e_all = big.tile([128, NCH, SPAN], bf16, tag="e_all", name="e_all")
        band_all = big.tile([128, NCH, L], bf16, tag="band_all", name="band_all")
        p_all = big.tile([128, NCH, L], f32, tag="p_all", name="p_all")
        den_all = big.tile([128, NCH], f32, tag="den_all", name="den_all")
        inv_all = big.tile([128, NCH], f32, tag="inv_all", name="inv_all")

        # load encodings in 4 chunked DMAs, interleaved with the remaining
        # one-hot blocks (FIFO order tuned so chunk-0 work starts earliest)
        for h in range(2):
            nc.sync.dma_start(
                out=enc_all[:, 2 * h:2 * (h + 1), :],
                in_=enc_v[:, 2 * h:2 * (h + 1), :],
            )
        for c0, cw in _oh_parts[1:]:
            nc.sync.dma_start(
                out=oh_t[:, :, c0:c0 + cw], in_=oh_v[:, :, c0:c0 + cw]
            )
        for h in range(2, 8):
            nc.sync.dma_start(
                out=enc_all[:, 2 * h:2 * (h + 1), :],
                in_=enc_v[:, 2 * h:2 * (h + 1), :],
            )

        # ---- gathers via one-hot matmul, emitted interleaved with the
        # group pipeline so each group's inputs come first in queue order ----
        kt_sb = [
            kt_pool.tile([128, TP], bf16, tag=f"kt{d}", name=f"kt_sb{d}")
            for d in range(4)
        ]
        vv_sb = [
            vv_pool.tile([128, D], bf16, tag=f"vv{j}", name=f"vv_sb{j}")
            for j in range(17)
        ]

        def emit_kt_block(tb):
            c0 = 512 * tb
            cw = min(512, TP - c0)
            for d in range(4):
                kps = pbig.tile([128, 512], f32, tag="pbig", name="kps")
                for v in range(2):
                    nc.tensor.matmul(
                        kps[:, :cw],
                        lhsT=m_sb[v][:, 128 * d:128 * (d + 1)],
                        rhs=oh_sb[v][:, c0:c0 + cw],
                        start=(v == 0),
                        stop=(v == 1),
                    )
                nc.scalar.activation(
                    out=kt_sb[d][:, c0:c0 + cw], in_=kps[:, :cw], func=Copy
                )

        def emit_vv_tile(j):
            r0 = 128 * j
            rw = min(128, TP - r0)  # 128, last tile 15
            vps = pbig.tile([128, 512], f32, tag="pbig", name="vps")
            for v in range(2):
                nc.tensor.matmul(
                    vps[:rw, :],
                    lhsT=oh_sb[v][:, r0:r0 + rw],
                    rhs=c_sb[v],
                    start=(v == 0),
                    stop=(v == 1),
                )
            nc.vector.tensor_copy(out=vv_sb[j][:rw, :], in_=vps[:rw, :])

        # a group covering chunks [g0, g0+gs) needs KT t-blocks up to
        # (128*(g0+gs)+142)//512 and Vv tiles up to g0+gs. Pre-emit group
        # 0's needs; trickle the rest a group ahead.
        _kt_done = [0]
        _vv_done = [0]

        def need_for_chunks(hi):
            kt_hi = min((128 * hi + SPAN - 1) // 512 + 1, 5)
            vv_hi = min(hi + 1, 17)
            while _kt_done[0] < kt_hi:
                emit_kt_block(_kt_done[0])
                _kt_done[0] += 1
            while _vv_done[0] < vv_hi:
                emit_vv_tile(_vv_done[0])
                _vv_done[0] += 1

        # ---- main pipeline: variable-size groups (finer at the tail) ----
        GROUPS = [(0, 4), (4, 4), (8, 4), (12, 4)]
        NG = len(GROUPS)
        need_for_chunks(GROUPS[0][0] + GROUPS[0][1])

        def emit_phase_a(g0_, gs_):
            for k in range(g0_, g0_ + gs_):
                t0 = 128 * k
                # enc -> bf16 on the otherwise-idle GPSIMD engine, then
                # PE-transpose at the bf16 rate (half the fp32 cost)
                qb_t = work.tile([128, 512], bf16, tag="qb", name="qb_t")
                nc.gpsimd.tensor_copy(out=qb_t, in_=enc_all[:, k, :])
                qps = pqt.tile([128, 512], bf16, tag="pqt", name="qps")
                for d in range(4):
                    nc.tensor.transpose(
                        out=qps[:, 128 * d:128 * (d + 1)],
                        in_=qb_t[:, 128 * d:128 * (d + 1)],
                        identity=idb_sb,
                    )
                qt_t = work.tile([128, 512], bf16, tag="qt", name="qt_t")
                nc.vector.tensor_copy(out=qt_t, in_=qps)

                # scores S[t, c] = sum_d QT[d, t] * KT[d, t0 + c]
                sps = ps_pool.tile([128, SPAN], f32, tag="ps", name="sps")
                for d in range(4):
                    nc.tensor.matmul(
                        sps,
                        lhsT=qt_t[:, 128 * d:128 * (d + 1)],
                        rhs=kt_sb[d][:, t0:t0 + SPAN],
                        start=(d == 0),
                        stop=(d == 3),
                    )
                # band mask (0 in band, -30000 outside), then exp -> bf16
                sm_t = work.tile([128, SPAN], f32, tag="sm", name="sm_t")
                nc.vector.tensor_add(out=sm_t, in0=sps, in1=msk_sb)
                # out-of-band entries are exp(-30000) = 0, so the free-dim
                # accumulator is exactly the softmax denominator -> phase C
                # never waits on the band round-trip
                nc.scalar.activation(
                    out=e_all[:, k, :], in_=sm_t, func=Exp,
                    accum_out=den_all[:, k:k + 1],
                )
            nc.vector.reciprocal(
                out=inv_all[:, g0_:g0_ + gs_],
                in_=den_all[:, g0_:g0_ + gs_],
            )

        # software-pipelined emission: phase A runs two groups ahead of B/C
        need_for_chunks(GROUPS[1][0] + GROUPS[1][1])
        emit_phase_a(*GROUPS[0])
        emit_phase_a(*GROUPS[1])
        for g, (g0, gs) in enumerate(GROUPS):
            ks = range(g0, g0 + gs)
            if g + 2 < NG:
                need_for_chunks(GROUPS[g + 2][0] + GROUPS[g + 2][1])
                emit_phase_a(*GROUPS[g + 2])

            # phase B: batched band extraction + softmax normalizers.
            # Rows are packed contiguously (stride SPAN); the diagonal of
            # each chunk tile is then a regular [[gs*SPAN+1],[SPAN],[1]] read
            # (band[p, c, l] sits at p*(gs*SPAN) + c*SPAN + p + l).
            scr_t = scr_pool.tile(
                [128 * GSZ * SPAN], bf16, tag="scr", name="scr_t", bufs=2
            )
            scr_ap = scr_t[:]
            scr_w_ap = bass.AP(
                tensor=scr_ap.tensor,
                offset=scr_ap.offset,
                ap=[[gs * SPAN, 128], [SPAN, gs], [1, SPAN]],
            )
            nc.sync.dma_start(
                out=scr_w_ap, in_=e_all[:, g0:g0 + gs, :]
            )
            band_src = bass.AP(
                tensor=scr_ap.tensor,
                offset=scr_ap.offset,
                ap=[[gs * SPAN + 1, 128], [SPAN, gs], [1, L]],
            )
            nc.scalar.dma_start(
                out=band_all[:, g0:g0 + gs, :], in_=band_src
            )
            for k in ks:
                nc.vector.tensor_scalar_mul(
                    out=p_all[:, k, :], in0=band_all[:, k, :],
                    scalar1=inv_all[:, k:k + 1],
                )
            nc.sync.dma_start(
                out=p_v[:, g0:g0 + gs, :],
                in_=p_all[:, g0:g0 + gs, :],
            )

            # phase C: E^T transpose + value matmul per chunk
            for k in ks:
                eps_t = pet.tile([128, 256], bf16, tag="pet", name="eps_t")
                nc.tensor.transpose(
                    out=eps_t[:, 0:128], in_=e_all[:, k, 0:128], identity=idb_sb
                )
                nc.tensor.transpose(
                    out=eps_t[0:15, 128:256], in_=e_all[:, k, 128:SPAN],
                    identity=idb_sb,
                )
                eta_t = work.tile([128, 128], bf16, tag="eta", name="eta_t")
                nc.vector.tensor_copy(out=eta_t, in_=eps_t[:, 0:128])
                etb_t = work.tile([15, 128], bf16, tag="etb", name="etb_t")
                nc.scalar.activation(
                    out=etb_t, in_=eps_t[0:15, 128:256], func=Copy
                )

                # numerator[t, :] = sum_c E[t, c] * Vv[t0 + c, :]
                nps = pbig.tile([128, 512], f32, tag="pbig", name="nps")
                nc.tensor.matmul(
                    nps, lhsT=eta_t, rhs=vv_sb[k], start=True, stop=False
                )
                nc.tensor.matmul(
                    nps, lhsT=etb_t, rhs=vv_sb[k + 1][0:15, :],
                    start=False, stop=True,
                )
                nc.scalar.activation(
                    out=comp_all[:, k, :], in_=nps, func=Copy,
                    scale=inv_all[:, k:k + 1],
                )

            # compressed half of the output for this group; finer writes in
            # the last groups shrink the drain tail
            step = 1
            for lo in range(g0, g0 + gs, step):
                nc.sync.dma_start(
                    out=out_v[:, lo:lo + step, 0:D],
                    in_=comp_all[:, lo:lo + step, :],
                )
            # passthrough half for this group (no consumers; fills DMA
            # slack; split so it never blocks a latency-critical DMA long)
            for lo in range(g0, g0 + gs):
                nc.sync.dma_start(
                    out=out_v[:, lo:lo + 1, D:2 * D],
                    in_=enc_all[:, lo:lo + 1, :],
                )

    _fix_excess_waits(nc)
    return nc


def _host_inputs(symbols, encodings, M, C):
    BW = 2 * D + 128
    FW = 128 + SPAN
    Mb = M.astype(BF16)
    Cb = C.astype(BF16)
    tt = np.arange(128)[:, None]
    cc = np.arange(SPAN)[None, :]
    mask = np.where((cc - tt >= 0) & (cc - tt <= L - 1), 0.0, -30000.0).astype(
        np.float32
    )
    cf = np.zeros((128, FW), dtype=np.float32)
    cf[:, 0:128] = np.eye(128, dtype=np.float32)
    cf[:, 128:128 + SPAN] = mask

    cb = np.zeros((2, 128, BW), dtype=BF16)
    for v in range(2):
        cb[v, :, 0:D] = Mb[128 * v:128 * (v + 1)]
        cb[v, :, D:2 * D] = Cb[128 * v:128 * (v + 1)]
    cb[0, :, 2 * D:2 * D + 128] = np.eye(128, dtype=BF16)

    pad_idx = np.clip(np.arange(TP) - (L - 1), 0, T - 1)
    vids = np.arange(V)[:, None]
    in_maps = []
    for b in range(N_CORES):
        sym_pad = np.asarray(symbols[b])[pad_idx]
        ohb = (sym_pad[None, :] == vids).astype(BF16).reshape(2, 128, TP)
        in_maps.append(
            {
                "enc": np.ascontiguousarray(encodings[b], dtype=np.float32),
                "cb": cb,
                "cf": cf,
                "oh": ohb,
            }
        )
    return in_maps


def kernel(symbols, encodings, M, C, _trace=False):
    from concourse.bass_utils import run_bass_kernel_spmd

    symbols = np.asarray(symbols)
    encodings = np.asarray(encodings)
    M = np.asarray(M, dtype=np.float32)
    C = np.asarray(C, dtype=np.float32)

    if "nc" not in _NC_CACHE:
        _NC_CACHE["nc"] = _build_nc()
    nc = _NC_CACHE["nc"]

    in_maps = _host_inputs(symbols, encodings, M, C)
    res = run_bass_kernel_spmd(
        nc, in_maps, list(range(N_CORES)), trace=_trace
    )
    out = np.stack([res.results[b]["out"] for b in range(N_CORES)]).astype(
        np.float32
    )
    p = np.stack([res.results[b]["p"] for b in range(N_CORES)]).astype(np.float32)
    if _trace:
        return (out, p), res
    return out, p
